# revision 5
# baseline (speedup 1.0000x reference)
"""Segment-gather-mean kernel for Trainium2 (8 NeuronCores), v4.

out[a] = mean over edges e with ancestors[e] == a of features[curr_nodes_idx[e]]

Design:
- Ancestors are dealt to (core, window, lane) slots by a degree-balanced
  snake so every (core, window) bin carries ~equal edge count: cross-core
  chunk boundaries stay tight and padding is <1%.
- Windows are processed in rotated order [48, 0, 1, ..., 47] so only one
  window's compute remains after the final gather lands (short tail).
- Per core, edges are split by node index (A: <32768, B: >=, the int16
  dma_gather limit), sorted by processing position, and packed into
  128-slot chunks with no per-window rounding (only stream tails pad).
- Gathers move fp16 feature rows over the 4 SWDGE queues round-robin
  (desc-gen parallelizes across queues, ~8.3 ns/row each; the Pool engine
  dispatches in strict rotation so aggregate ~2.1 ns/row). Head and tail
  gathers are small to cut pipeline fill/drain latency; the first two
  gathers' indices ride a separate tiny DRAM tensor so nothing gates them.
- A chunk may span two adjacent positions; each window's matmuls cover a
  static cross-core chunk range and the fp16 one-hot (is_equal against an
  iota table, seg = position-parity*128 + lane) masks foreign slots. Host
  asserts same-parity positions never share a chunk.
- PE accumulates psum fp32; scalar engine scales by 1/count; rows stream
  out via HWDGE.
"""

import math
import os
import sys

sys.path.insert(0, "/opt/trn_rl_repo")

import numpy as np

P = 128
D = 128
N_OUT = 50000
N_NODES = 50000
N_CORES = 8
NSEG = N_OUT // N_CORES          # segments per core (6250)
W = math.ceil(NSEG / P)          # seg windows per core (49)
LAST_ROWS = NSEG - (W - 1) * P   # rows in the final hw window (106)
SPLIT = 32768                    # int16 gather index limit
GCH = 17                         # steady-state chunks per gather (2176 rows)
N_HEAD = 2                       # gathers per stream whose idx ride the
                                 # dedicated head tensor

PROC = [W - 1] + list(range(W - 1))          # processing order of hw windows
POSW = [0] * W                               # hw window -> position
for _p, _w in enumerate(PROC):
    POSW[_w] = _p

TRACE = os.environ.get("KERNEL_TRACE", "0") == "1"
last_results = None

_nc_cache = {}


def _plan_gathers(n_chunks):
    """Gather sizes: small head (pipeline fill) and tail (drain), 17 mid."""
    head = [5, 5, 7]
    tail = [4, 2, 2]
    if n_chunks <= sum(head) + sum(tail):
        sizes = []
        rem = n_chunks
        while rem > 0:
            k = min(8, rem)
            sizes.append(k)
            rem -= k
    else:
        sizes = list(head)
        rem = n_chunks - sum(head) - sum(tail)
        while rem > GCH:
            sizes.append(GCH)
            rem -= GCH
        if rem:
            sizes.append(rem)
        sizes += tail
    out = []
    j = 0
    for k in sizes:
        out.append((j, k))
        j += k
    return out


def _build_nc(CA, CB, rngA, rngB, max_range):
    """CA/CB: chunks per stream. rngA/rngB: per-position [lo, hi) ranges."""
    import concourse.bacc as bacc
    import concourse.mybir as mybir
    from concourse.tile import TileContext, add_dep_helper

    gA = _plan_gathers(CA)
    gB = _plan_gathers(CB)
    colsA = CA * 8           # idx cols per stream (128 idx -> 8 cols of 16)
    colsB = CB * 8
    hA = sum(k for _, k in gA[:N_HEAD]) * 8     # head idx cols
    hB = sum(k for _, k in gB[:N_HEAD]) * 8

    nc = bacc.Bacc("TRN2", target_bir_lowering=False, debug=False,
                   num_devices=N_CORES, num_swdge_queues=4)
    feat = nc.dram_tensor("feat", [N_NODES, D], mybir.dt.float16,
                          kind="ExternalInput")
    idxh = nc.dram_tensor("idxh", [P, hA + hB], mybir.dt.int16,
                          kind="ExternalInput")
    idx = nc.dram_tensor("idx", [P, colsA + colsB], mybir.dt.int16,
                         kind="ExternalInput")
    seg = nc.dram_tensor("seg", [P, CA + CB], mybir.dt.float16,
                         kind="ExternalInput")
    iota = nc.dram_tensor("iota", [P, max_range, 256], mybir.dt.float16,
                          kind="ExternalInput")
    recip = nc.dram_tensor("recip", [P, W], mybir.dt.float32,
                           kind="ExternalInput")
    out = nc.dram_tensor("out", [NSEG, D], mybir.dt.float32,
                         kind="ExternalOutput")

    def first_pos(ranges, j0, k):
        for p in range(W):
            lo, hi = ranges[p]
            if lo < j0 + k and hi > j0:
                return p
        return W

    sched = []   # (first_pos, stream, gather_idx, j0, k)
    for gi, (j0, k) in enumerate(gA):
        sched.append((first_pos(rngA, j0, k), 0, gi, j0, k))
    for gi, (j0, k) in enumerate(gB):
        sched.append((first_pos(rngB, j0, k), 1, gi, j0, k))
    sched.sort(key=lambda t: (t[0], t[2], t[1]))

    with TileContext(nc) as tc:
        with (
            tc.tile_pool(name="const", bufs=1) as cpool,
            tc.tile_pool(name="gath", bufs=26) as gpool,
            tc.tile_pool(name="oh", bufs=4) as ohpool,
            tc.tile_pool(name="psum", bufs=4, space="PSUM") as ppool,
            tc.tile_pool(name="osb", bufs=4) as opool,
        ):
            idxh_sb = cpool.tile([P, hA + hB], mybir.dt.int16)
            idx_sb = cpool.tile([P, colsA + colsB], mybir.dt.int16)
            seg_sb = cpool.tile([P, CA + CB], mybir.dt.float16)
            iota_sb = cpool.tile([P, max_range, 256], mybir.dt.float16)
            recip_sb = cpool.tile([P, W], mybir.dt.float32)

            # head idx first (tiny, gates the first gathers), then seg+iota
            # (gate the first one-hots/matmuls), then bulk idx, then recip
            reg_cache = {}

            def reg_for(v):
                if v not in reg_cache:
                    reg_cache[v] = nc.gpsimd.to_reg(v)
                return reg_cache[v]

            nc.sync.dma_start(idxh_sb[:], idxh[:])
            nc.sync.dma_start(recip_sb[:], recip[:])
            nc.sync.dma_start(seg_sb[:], seg[:])
            # bulk idx in two slices per stream: a small early slice covers
            # the next ~8 gathers so they never wait on the 1MB remainder
            sA = min(hA + 64 * 8, colsA)
            sB = min(hB + 32 * 8, colsB)
            nc.sync.dma_start(idx_sb[:, hA:sA], idx[:, hA:sA])
            nc.sync.dma_start(idx_sb[:, colsA + hB:colsA + sB],
                              idx[:, colsA + hB:colsA + sB])
            if sA < colsA:
                nc.sync.dma_start(idx_sb[:, sA:colsA], idx[:, sA:colsA])
            if sB < colsB:
                nc.sync.dma_start(idx_sb[:, colsA + sB:],
                                  idx[:, colsA + sB:])
            nc.sync.dma_start(iota_sb[:], iota[:])

            chunk_tile_A = {}
            chunk_tile_B = {}
            prev_gather = None
            n_emitted = 0
            sched_pos = 0

            def emit_gather(stream, gi, j0, k):
                nonlocal prev_gather, n_emitted
                t = gpool.tile([P, k, D], mybir.dt.float16)
                if stream == 0:
                    src = feat[:SPLIT, :]
                    if gi < N_HEAD:
                        isb = idxh_sb[:, j0 * 8: j0 * 8 + k * 8]
                    else:
                        isb = idx_sb[:, j0 * 8: (j0 + k) * 8]
                else:
                    src = feat[SPLIT:, :]
                    if gi < N_HEAD:
                        isb = idxh_sb[:, hA + j0 * 8: hA + (j0 + k) * 8]
                    else:
                        isb = idx_sb[:, colsA + j0 * 8: colsA + (j0 + k) * 8]
                g_i = nc.gpsimd.dma_gather(
                    t[:], src, isb,
                    k * P, reg_for(k * P), D, single_packet=False,
                    queue_num=n_emitted % 4)
                if prev_gather is not None:
                    add_dep_helper(g_i.ins, prev_gather.ins, sync=False,
                                   reason="swdge lane/queue congruence")
                prev_gather = g_i
                n_emitted += 1
                mapping = chunk_tile_A if stream == 0 else chunk_tile_B
                for jj in range(k):
                    mapping[j0 + jj] = (t, jj)

            for p in range(W):
                while sched_pos < len(sched) and sched[sched_pos][0] <= p:
                    _, st, gi, j0, k = sched[sched_pos]
                    emit_gather(st, gi, j0, k)
                    sched_pos += 1

                w = PROC[p]
                par = (p % 2) * 128
                loA, hiA = rngA[p]
                loB, hiB = rngB[p]
                lenA = hiA - loA
                lenB = hiB - loB
                n_mm = lenA + lenB
                assert n_mm > 0

                ohA = ohB = None
                if lenA:
                    ohA = ohpool.tile([P, lenA, P], mybir.dt.float16)
                    nc.vector.tensor_tensor(
                        out=ohA[:],
                        in0=iota_sb[:, :lenA, par:par + P],
                        in1=seg_sb[:, loA:hiA].to_broadcast([P, lenA, P]),
                        op=mybir.AluOpType.is_equal,
                    )
                if lenB:
                    ohB = ohpool.tile([P, lenB, P], mybir.dt.float16)
                    nc.vector.tensor_tensor(
                        out=ohB[:],
                        in0=iota_sb[:, :lenB, par:par + P],
                        in1=seg_sb[:, CA + loB:CA + hiB].to_broadcast(
                            [P, lenB, P]),
                        op=mybir.AluOpType.is_equal,
                    )

                ps = ppool.tile([P, D], mybir.dt.float32, space="PSUM")
                mm = 0
                for jj in range(lenA):
                    t, tj = chunk_tile_A[loA + jj]
                    nc.tensor.matmul(
                        ps[:], lhsT=ohA[:, jj, :], rhs=t[:, tj, :],
                        start=(mm == 0), stop=(mm == n_mm - 1))
                    mm += 1
                for jj in range(lenB):
                    t, tj = chunk_tile_B[loB + jj]
                    nc.tensor.matmul(
                        ps[:], lhsT=ohB[:, jj, :], rhs=t[:, tj, :],
                        start=(mm == 0), stop=(mm == n_mm - 1))
                    mm += 1

                osb = opool.tile([P, D], mybir.dt.float32)
                nc.scalar.activation(
                    osb[:], ps[:], mybir.ActivationFunctionType.Copy,
                    scale=recip_sb[:, w:w + 1])
                rows = P if w < W - 1 else LAST_ROWS
                nc.scalar.dma_start(out[w * P: w * P + rows, :],
                                    osb[:rows, :])

    nc.compile()
    return nc


def _assign_slots(anc):
    """Degree-balanced snake: ancestor -> (core, local_id in [0, 6250))."""
    deg = np.bincount(anc, minlength=N_OUT)
    order = np.argsort(-deg, kind="stable")

    core_of = np.empty(N_OUT, np.int32)
    idx = np.arange(N_OUT)
    rounds = idx // N_CORES
    lanes = idx % N_CORES
    lanes = np.where(rounds % 2 == 1, N_CORES - 1 - lanes, lanes)
    core_of[order] = lanes

    caps = np.full(W, P, np.int64)
    caps[W - 1] = LAST_ROWS
    # window sequence for one core's degree-sorted members (snake over
    # windows, respecting capacities) -- same for every core
    fill = np.zeros(W, np.int64)
    wseq = []
    r = 0
    while len(wseq) < NSEG:
        ws = range(W) if r % 2 == 0 else range(W - 1, -1, -1)
        for w_ in ws:
            if fill[w_] < caps[w_]:
                wseq.append(w_ * P + fill[w_])
                fill[w_] += 1
        r += 1
    wseq = np.array(wseq[:NSEG], np.int64)

    lid_of = np.empty(N_OUT, np.int64)
    for c in range(N_CORES):
        members = order[core_of[order] == c]     # degree-sorted
        lid_of[members] = wseq
    return core_of, lid_of


def _prep_core(lid, n_l, CA, CB):
    """Build per-core idx/seg arrays given static chunk counts CA/CB.

    Streams are sorted by processing position; seg = (pos%2)*128 + lane.
    """
    posw = np.array(POSW, np.int64)
    low = n_l < SPLIT
    out_idx = np.zeros((P, (CA + CB) * 8), np.int16)
    out_seg = np.full((P, CA + CB), -1.0, np.float16)

    for sel, off0, C, col0, segcol0 in (
        (low, 0, CA, 0, 0),
        (~low, SPLIT, CB, CA * 8, CA),
    ):
        a = lid[sel]
        n = (n_l[sel] - off0).astype(np.int16)
        key = posw[a // P] * P + (a % P)
        o = np.argsort(key, kind="stable")
        key = key[o]
        n = n[o]
        m = C * P
        idx_flat = np.zeros(m, np.int16)
        idx_flat[:len(n)] = n
        seg_flat = np.full(m, -1.0, np.float16)
        seg_flat[:len(key)] = (((key // P) % 2) * P + (key % P)).astype(
            np.float16)
        wrapped = idx_flat.reshape(C * 8, 16).T      # [16, C*8]
        out_idx[:, col0:col0 + C * 8] = np.tile(wrapped, (8, 1))
        out_seg[:, segcol0:segcol0 + C] = seg_flat.reshape(C, P).T

    return out_idx, out_seg


def _prepare(features, nodes, anc):
    core_of, lid_of = _assign_slots(anc)
    core_e = core_of[anc]
    lid_e = lid_of[anc]
    posw = np.array(POSW, np.int64)

    per_core = []
    cntsA = np.zeros((N_CORES, W + 1), np.int64)   # cumulative, by position
    cntsB = np.zeros((N_CORES, W + 1), np.int64)
    for c in range(N_CORES):
        m = core_e == c
        lid = lid_e[m]
        n_l = nodes[m]
        per_core.append((lid, n_l))
        low = n_l < SPLIT
        p_l = posw[lid // P]
        cntsA[c, 1:] = np.cumsum(np.bincount(p_l[low], minlength=W))
        cntsB[c, 1:] = np.cumsum(np.bincount(p_l[~low], minlength=W))

    CA = int(max(math.ceil(cntsA[c, W] / P) for c in range(N_CORES)))
    CB = int(max(math.ceil(cntsB[c, W] / P) for c in range(N_CORES)))

    def ranges(cnts, C):
        rng = []
        for p in range(W):
            lo = min(int(cnts[c, p] // P) for c in range(N_CORES))
            hi = max(int(math.ceil(cnts[c, p + 1] / P))
                     for c in range(N_CORES))
            hi = min(hi, C)
            lo = min(lo, hi)
            if hi == lo:
                hi = min(lo + 1, C)
                lo = hi - 1
            rng.append((lo, hi))
        for p in range(W - 2):
            assert rng[p][1] <= rng[p + 2][0], (p, rng[p], rng[p + 2])
        return rng

    rngA = ranges(cntsA, CA)
    rngB = ranges(cntsB, CB)
    max_range = max(max(hi - lo for lo, hi in rngA),
                    max(hi - lo for lo, hi in rngB))

    cnt = np.bincount(anc, minlength=N_OUT).astype(np.float32)
    recip_all = 1.0 / np.maximum(cnt, 1.0)

    iota = np.broadcast_to(
        np.arange(256, dtype=np.float16)[None, None, :],
        (P, max_range, 256)).copy()

    feat16 = features.astype(np.float16)

    gA = _plan_gathers(CA)
    gB = _plan_gathers(CB)
    hA = sum(k for _, k in gA[:N_HEAD]) * 8
    hB = sum(k for _, k in gB[:N_HEAD]) * 8

    slot_e = core_of.astype(np.int64) * NSEG + lid_of   # ancestor -> out row
    in_maps = []
    for c in range(N_CORES):
        lid, n_l = per_core[c]
        idx_np, seg_np = _prep_core(lid, n_l, CA, CB)
        idxh_np = np.concatenate(
            [idx_np[:, :hA], idx_np[:, CA * 8:CA * 8 + hB]], axis=1)
        idxh_np = np.ascontiguousarray(idxh_np)
        mc = core_of == c
        rr = np.ones(W * P, np.float32)
        rr[lid_of[mc]] = recip_all[mc]
        r_sb = np.ascontiguousarray(rr.reshape(W, P).T)
        in_maps.append({
            "feat": feat16,
            "idxh": idxh_np,
            "idx": idx_np,
            "seg": seg_np,
            "iota": iota,
            "recip": r_sb,
        })
    return CA, CB, rngA, rngB, max_range, in_maps, slot_e


def kernel(**inputs):
    from concourse.bass_utils import run_bass_kernel_spmd

    features = np.asarray(inputs["features"], dtype=np.float32)
    nodes = np.asarray(inputs["curr_nodes_idx"]).astype(np.int64)
    anc = np.asarray(inputs["ancestors"]).astype(np.int64)
    uall = np.asarray(inputs["uall_ancestors_idx"]).astype(np.int64)

    CA, CB, rngA, rngB, max_range, in_maps, slot_e = _prepare(
        features, nodes, anc)

    key = (CA, CB, tuple(rngA), tuple(rngB))
    if key not in _nc_cache:
        _nc_cache[key] = _build_nc(CA, CB, rngA, rngB, max_range)
    nc = _nc_cache[key]

    res = run_bass_kernel_spmd(nc, in_maps, core_ids=list(range(N_CORES)),
                               trace=TRACE)
    global last_results
    last_results = res
    rows = np.concatenate([res.results[c]["out"] for c in range(N_CORES)],
                          axis=0)
    out = np.zeros((N_OUT, D), np.float32)
    out[uall] = rows[slot_e]
    return out
